# revision 1
# baseline (speedup 1.0000x reference)
"""Trainium2 Bass kernel for the GCNN layer (nn_GCNNLayer_71536975282326).

out = relu( einsum('nd,nde->ne', x, W_pos) + b_pos
            + einsum('nre,nr->ne', einsum('nd,rde->nre', x, W_dep), counts)
            + counts @ b_dep )
with counts[n,r] = #edges (token n, type r).

Strategy (8 NeuronCores, SPMD, one program):
  - Shard the R=92 W_dep stack across cores (12 slots/core, zero-padded) and
    the N=150 W_pos stack across cores (19 slots/core, zero-padded).
  - All heavy matmuls run in float32r (fp32 storage, ~4x PE rate, measured
    ~1.4e-4 scale-relative error end to end).
  - Dep accumulation is transposed, out_T[e, n], in 4 PSUM banks (two 256-wide
    token regions per bank — f32r needs a >=256-wide moving operand for full
    rate). Moving operand is the host-prescaled (counts[:,r]*x)^T.
  - Self term: per token, 16 M=1 matmuls (W_pos[n_j] chunks moving 512-wide)
    into partition 0 of a small PSUM tile; the row is bounced through SBUF
    (engines cannot address partitions at j>0) into the AllGather input.
    Self units run FIRST so the AllGather overlaps the dep DMA stream.
  - The gathered [152, 1024] self matrix ([token, e]) is transposed back to
    [e, token] ON THE PE via identity matmuls into the freed PSUM banks —
    a strided-DMA transpose would degrade to 4-byte packets and flood the
    DMA engines (measured: 158K single-element packets, +300us).
  - Bias: one K=32 f32 matmul per e-chunk: lhsT rows = [b_dep slice ; b_pos
    rows], rhs = [counts slice^T ; one-hot placing token n_j at column n_j].
  - ReduceScatter the [1024,150] main partial (core k receives e-chunk k);
    each core adds its self_T chunk (partition_id-driven dynamic slice),
    applies relu, and outputs its [128,150] chunk; the host concatenates the
    8 chunks and transposes.
  - Every dma_start stays <=256 packets (HWDGE ring depth) and triggers are
    spread across the sync/gpsimd/scalar/vector queues.
"""

import numpy as np

import concourse.bass as bass
import concourse.tile as tile
from concourse import bacc, mybir
from concourse.bass_utils import run_bass_kernel_spmd

N, D, R = 150, 1024, 92
NCORES = 8
P = 128
DC = D // P            # 8 contraction (d) chunks
EC = D // P            # 8 output (e) chunks
NB = EC // 2           # 4 main psum banks, two e-chunks each
NPAD = 256             # token axis padded so f32r moving >= 256
DEP_SLOTS = 12         # crep slots: 11 full types + 1 half type per core
DEP_FULL = 11          # full dep types per core (8*11 = 88)
HC = 4                 # chunks in the half slot (types 88..91 split row-wise
                       # across core pairs; partial sums meet in the reduce)
SELF_SLOTS = 19        # ceil(150/8)
NSELF = NCORES * SELF_SLOTS  # 152 gathered self rows
KAUG = 32              # 12 dep-count rows + 19 one-hot rows + 1 pad
F32 = mybir.dt.float32
F32R = mybir.dt.float32r

DEP_SPLIT = [12, 12, 12, 12, 11, 11, 11, 11]
DEP_STARTS = np.concatenate([[0], np.cumsum(DEP_SPLIT)])

_PROG = None


def _build_program():
    nc = bacc.Bacc("TRN2", target_bir_lowering=False, debug=False, num_devices=NCORES)

    wdep = nc.dram_tensor("wdep", [DEP_FULL, D, D], F32R, kind="ExternalInput")
    whalf = nc.dram_tensor("whalf", [HC * P, D], F32R, kind="ExternalInput")
    # this core's half of X^T chunks for the split type (chunks 0-3 or 4-7)
    xtf2 = nc.dram_tensor("xtf2", [P, HC * N], F32R, kind="ExternalInput")
    wpos = nc.dram_tensor("wpos", [SELF_SLOTS, D, D], F32R, kind="ExternalInput")
    # full X^T in tile layout [p, c*N+n] plus counts replicated across
    # partitions: the counts-scaled moving operand is built on the idle DVE
    # instead of being streamed from HBM (saves 7.4MB/core of xs traffic)
    xtf = nc.dram_tensor("xtf", [P, DC * N], F32R, kind="ExternalInput")
    crep = nc.dram_tensor("crep", [P, DEP_SLOTS * N], F32R, kind="ExternalInput")
    xtl = nc.dram_tensor("xtl", [DC, P, SELF_SLOTS], F32R, kind="ExternalInput")
    baug = nc.dram_tensor("baug", [KAUG, D], F32, kind="ExternalInput")
    caug = nc.dram_tensor("caug", [KAUG, NPAD], F32, kind="ExternalInput")
    # identity used to PE-transpose the gathered self rows: ident[g, j, n] = 1
    # iff n == 128*g + j
    ident = nc.dram_tensor("ident", [2, P, NPAD], F32R, kind="ExternalInput")
    # per-core output: this core's 128-row e-chunk of out_T (host assembles)
    out_T = nc.dram_tensor("out_T", [P, N], F32, kind="ExternalOutput")

    groups = [list(range(NCORES))]

    with tile.TileContext(nc) as tc:
        with (
            tc.tile_pool(name="constp", bufs=1) as constp,
            tc.tile_pool(name="mainps", bufs=1, space=bass.MemorySpace.PSUM) as mainps,
            tc.tile_pool(name="selfps", bufs=4, space=bass.MemorySpace.PSUM) as selfps,
            tc.tile_pool(name="dram", bufs=1, space="DRAM") as dram,
            tc.tile_pool(name="fin", bufs=3) as fin,
        ):
            xtl_t = constp.tile([P, DC * SELF_SLOTS], F32R)
            nc.gpsimd.dma_start(
                out=xtl_t.rearrange("p (c j) -> p c j", c=DC),
                in_=xtl[:].rearrange("c p j -> p c j"),
            )
            baug_t = constp.tile([KAUG, D], F32)
            nc.gpsimd.dma_start(out=baug_t[:], in_=baug[:])
            caug_t = constp.tile([KAUG, NPAD], F32)
            nc.gpsimd.dma_start(out=caug_t[:], in_=caug[:])
            xtf_t = constp.tile([P, DC * N], F32R)
            nc.scalar.dma_start(out=xtf_t[:], in_=xtf[:])
            xtf2_t = constp.tile([P, HC * N], F32R)
            nc.scalar.dma_start(out=xtf2_t[:], in_=xtf2[:])
            crep_t = constp.tile([P, DEP_SLOTS * N], F32R)
            nc.scalar.dma_start(out=crep_t[:], in_=crep[:])
            ident_t = constp.tile([P, 2 * NPAD], F32R)
            nc.gpsimd.dma_start(
                out=ident_t.rearrange("p (g n) -> p g n", g=2),
                in_=ident[:].rearrange("g p n -> p g n"),
            )

            accs = [
                mainps.tile([P, 2 * NPAD], F32, name=f"acc{b}", tag=f"acc{b}")
                for b in range(NB)
            ]
            # Bias matmuls first: the single start=True per main PSUM bank (the
            # second region's first-touch rides the bank's pending-zero state).
            for b in range(NB):
                for h in range(2):
                    nc.tensor.matmul(
                        accs[b][:, h * NPAD : h * NPAD + NPAD],
                        baug_t[:, (2 * b + h) * P : (2 * b + h + 1) * P],
                        caug_t[:],
                        start=(h == 0),
                        stop=False,
                    )

            stream_pools = (
                tc.tile_pool(name="wpool", bufs=4),
                tc.tile_pool(name="xspool", bufs=1),
            )
            wpool = stream_pools[0].__enter__()
            xspool = stream_pools[1].__enter__()

            def load_w(src, u, engs):
                wt = wpool.tile([P, DC * D], F32R, tag="w", name=f"w{u}")
                src3 = src.rearrange("(c p) e -> p c e", p=P)
                wt3 = wt.rearrange("p (c e) -> p c e", c=DC)
                for g in range(4):
                    engs[g % len(engs)].dma_start(
                        out=wt3[:, 2 * g : 2 * g + 2, :],
                        in_=src3[:, 2 * g : 2 * g + 2, :],
                    )
                return wt

            # ---- self phase: 19 tokens, M=1 row formulation ----
            ar_self_in = dram.tile([SELF_SLOTS, D], F32R)
            ar_self_out = dram.tile([NCORES, SELF_SLOTS, D], F32R, addr_space="Shared")
            for j in range(SELF_SLOTS):
                wt = load_w(wpos[j], f"s{j}", [nc.sync] if j % 2 == 0 else [nc.gpsimd])
                for eh in range(2):
                    st = selfps.tile([1, 512], F32, tag="sp", name=f"sp{j}_{eh}")
                    for c in range(DC):
                        nc.tensor.matmul(
                            st[:],
                            xtl_t[:, c * SELF_SLOTS + j : c * SELF_SLOTS + j + 1],
                            wt[:, c * D + eh * 512 : c * D + (eh + 1) * 512],
                            start=(c == 0),
                            stop=(c == DC - 1),
                        )
                    # ACT copy with f32r output = the "rounding" producer the
                    # BIR verifier wants for downstream f32r matmuls
                    sx = fin.tile([1, 512], F32R, tag="sx", name=f"sx{j}_{eh}")
                    nc.scalar.copy(out=sx[:], in_=st[:])
                    nc.scalar.dma_start(
                        out=ar_self_in[j : j + 1, eh * 512 : (eh + 1) * 512],
                        in_=sx[:],
                    )

            nc.gpsimd.collective_compute(
                "AllGather", mybir.AluOpType.bypass,
                replica_groups=groups, ins=[ar_self_in.opt()], outs=[ar_self_out.opt()],
            )

            # ---- dep phase: 12 type slots ----
            # xs tiles are pre-zeroed once; DMA refills only the first 150 of
            # each 256-wide chunk region, the zero padding is never rewritten.
            NXS = 3
            xsts = [xspool.tile([P, DC * NPAD], F32R, tag=f"xs{i}", name=f"xs{i}") for i in range(NXS)]
            for t in xsts:
                nc.vector.memset(t[:].bitcast(F32), 0.0)
            for i in range(DEP_FULL):
                # the final units' loads fan out across all trigger queues so
                # the stream tail drains at full rate instead of one queue
                if i >= DEP_FULL - 2:
                    w_engs = [nc.sync, nc.gpsimd, nc.scalar]
                else:
                    w_engs = [nc.sync] if i % 2 == 0 else [nc.gpsimd]
                wt = load_w(wdep[i], f"d{i}", w_engs)
                xst = xsts[i % NXS]
                for c in range(DC):
                    nc.vector.tensor_mul(
                        xst[:, c * NPAD : c * NPAD + N],
                        xtf_t[:, c * N : (c + 1) * N],
                        crep_t[:, i * N : (i + 1) * N],
                    )
                for c in range(DC):
                    for ec in range(EC):
                        b, h = divmod(ec, 2)
                        nc.tensor.matmul(
                            accs[b][:, h * NPAD : h * NPAD + NPAD],
                            wt[:, c * D + ec * P : c * D + (ec + 1) * P],
                            xst[:, c * NPAD : (c + 1) * NPAD],
                            start=False,
                            stop=False,
                        )

            # half slot: 4 chunks of the split type (this core's row-half)
            wth = wpool.tile([P, HC * D], F32R, tag="w", name="whalf")
            srcH = whalf[:].rearrange("(c p) e -> p c e", p=P)
            wth3 = wth.rearrange("p (c e) -> p c e", c=HC)
            for g in range(2):
                (nc.sync, nc.gpsimd)[g].dma_start(
                    out=wth3[:, 2 * g : 2 * g + 2, :],
                    in_=srcH[:, 2 * g : 2 * g + 2, :],
                )
            xsth = xsts[DEP_FULL % NXS]
            for c in range(HC):
                nc.vector.tensor_mul(
                    xsth[:, c * NPAD : c * NPAD + N],
                    xtf2_t[:, c * N : (c + 1) * N],
                    crep_t[:, DEP_FULL * N : (DEP_FULL + 1) * N],
                )
            for c in range(HC):
                for ec in range(EC):
                    b, h = divmod(ec, 2)
                    nc.tensor.matmul(
                        accs[b][:, h * NPAD : h * NPAD + NPAD],
                        wth[:, c * D + ec * P : c * D + (ec + 1) * P],
                        xsth[:, c * NPAD : (c + 1) * NPAD],
                        start=False,
                        stop=c == HC - 1 and h == 1,
                    )

            stream_pools[1].__exit__(None, None, None)
            stream_pools[0].__exit__(None, None, None)

            # ---- evacuate + ReduceScatter (core k receives e-chunk k) ----
            ar_main_in = dram.tile([D, N], F32)
            rs_out = dram.tile([P, N], F32)
            for b in range(NB):
                ev = fin.tile([P, 2 * NPAD], F32, tag="ev", name=f"ev{b}")
                nc.vector.tensor_copy(ev[:], accs[b][:])
                for h in range(2):
                    nc.sync.dma_start(
                        out=ar_main_in[(2 * b + h) * P : (2 * b + h + 1) * P, :],
                        in_=ev[:, h * NPAD : h * NPAD + N],
                    )
            nc.gpsimd.collective_compute(
                "ReduceScatter", mybir.AluOpType.add,
                replica_groups=groups, ins=[ar_main_in.opt()], outs=[rs_out.opt()],
            )

            # ---- PE-transpose the gathered self rows into the freed banks ----
            # self_all is [(core k, j) = token 19k+j, e]; we need [e, token].
            # out_T_chunk[e, n] = sum_j self[jg*128+j, e] * ident[jg][j, n]
            tailp_cm = tc.tile_pool(name="tailp", bufs=1)
            tailp = tailp_cm.__enter__()
            sj0 = tailp.tile([P, D], F32R, tag="sj0")
            sj1 = tailp.tile([NSELF - P, D], F32R, tag="sj1")
            sflat = ar_self_out[:].rearrange("k j e -> (k j) e")
            nc.gpsimd.dma_start(out=sj0[:], in_=sflat[0:P, :])
            nc.gpsimd.dma_start(out=sj1[:], in_=sflat[P:NSELF, :])
            for ec in range(EC):
                b, h = divmod(ec, 2)
                nc.tensor.matmul(
                    accs[b][:, h * NPAD : h * NPAD + NPAD],
                    sj0[:, ec * P : (ec + 1) * P],
                    ident_t[:, 0:NPAD],
                    start=(h == 0),
                    stop=False,
                )
                nc.tensor.matmul(
                    accs[b][:, h * NPAD : h * NPAD + NPAD],
                    sj1[:, ec * P : (ec + 1) * P],
                    ident_t[0 : NSELF - P, NPAD : 2 * NPAD],
                    start=False,
                    stop=(h == 1),
                )

            # ---- final combine (own e-chunk only): out_chunk = relu(rs + self_T) ----
            # selfT for ALL chunks sits in PSUM (the transpose is cheap and
            # keeps the program SPMD-uniform); this core's chunk is selected
            # with a partition_id-driven dynamic slice.
            selfT_sb = tailp.tile([P, NB * 2 * NPAD], F32, tag="sT")
            for b in range(NB):
                nc.vector.tensor_copy(
                    selfT_sb[:, b * 2 * NPAD : (b + 1) * 2 * NPAD], accs[b][:]
                )
            pid = nc.vector.partition_id()
            col0 = pid * NPAD
            mc = fin.tile([P, N], F32, tag="mc")
            nc.gpsimd.dma_start(out=mc[:], in_=rs_out[:])
            oc = fin.tile([P, N], F32, tag="oc")
            nc.vector.scalar_tensor_tensor(
                oc[:], mc[:], 0.0, selfT_sb[:, bass.ds(col0, N)],
                mybir.AluOpType.add, mybir.AluOpType.add,
            )
            nc.vector.tensor_scalar_max(oc[:], oc[:], 0.0)
            nc.sync.dma_start(out=out_T[:], in_=oc[:])
            tailp_cm.__exit__(None, None, None)

    nc.compile()
    return nc


def _get_program():
    global _PROG
    if _PROG is None:
        _PROG = _build_program()
    return _PROG


def _prepare_in_maps(x, W_pos, b_pos, W_dep, b_dep, edge_token, edge_type):
    x = np.ascontiguousarray(np.asarray(x, dtype=np.float32))
    W_pos = np.asarray(W_pos, dtype=np.float32)
    b_pos = np.asarray(b_pos, dtype=np.float32)
    W_dep = np.asarray(W_dep, dtype=np.float32)
    b_dep = np.asarray(b_dep, dtype=np.float32)
    edge_token = np.asarray(edge_token)
    edge_type = np.asarray(edge_type)

    counts = np.zeros((N, R), np.float32)
    np.add.at(counts, (edge_token, edge_type), 1.0)
    xT = np.ascontiguousarray(x.T)  # [D, N]
    xtf_np = np.ascontiguousarray(
        xT.reshape(DC, P, N).transpose(1, 0, 2).reshape(P, DC * N)
    )

    ident_np = np.zeros((2, P, NPAD), np.float32)
    for g in range(2):
        for j in range(P):
            n = g * P + j
            if n < NPAD:
                ident_np[g, j, n] = 1.0

    in_maps = []
    for k in range(NCORES):
        r0 = DEP_FULL * k
        stype = NCORES * DEP_FULL + k // 2   # split type for this core pair
        lower = k % 2 == 0                   # even core: rows 0:512 / chunks 0:4
        t0 = SELF_SLOTS * k
        t1 = min(t0 + SELF_SLOTS, N)
        nt = t1 - t0

        wdep_k = np.ascontiguousarray(W_dep[r0 : r0 + DEP_FULL])
        whalf_k = np.ascontiguousarray(
            W_dep[stype][0 : HC * P] if lower else W_dep[stype][HC * P : D]
        )
        xtf2_k = np.ascontiguousarray(
            xtf_np[:, 0 : HC * N] if lower else xtf_np[:, HC * N : 2 * HC * N]
        )
        wpos_k = np.zeros((SELF_SLOTS, D, D), np.float32)
        wpos_k[:nt] = W_pos[t0:t1]

        crep_k = np.zeros((P, DEP_SLOTS * N), np.float32)
        crep_k[:, 0 : DEP_FULL * N] = counts[:, r0 : r0 + DEP_FULL].T.reshape(
            1, DEP_FULL * N
        )
        crep_k[:, DEP_FULL * N :] = counts[:, stype].reshape(1, N)

        xtl_k = np.zeros((DC, P, SELF_SLOTS), np.float32)
        xtl_k[:, :, :nt] = xT[:, t0:t1].reshape(DC, P, nt)

        baug_k = np.zeros((KAUG, D), np.float32)
        baug_k[:DEP_FULL] = b_dep[r0 : r0 + DEP_FULL]
        baug_k[DEP_SLOTS : DEP_SLOTS + nt] = b_pos[t0:t1]

        caug_k = np.zeros((KAUG, NPAD), np.float32)
        caug_k[:DEP_FULL, 0:N] = counts[:, r0 : r0 + DEP_FULL].T
        if lower:
            # split type's bias is counted exactly once, on the even core
            baug_k[DEP_FULL] = b_dep[stype]
            caug_k[DEP_FULL, 0:N] = counts[:, stype]
        for j in range(nt):
            caug_k[DEP_SLOTS + j, t0 + j] = 1.0

        in_maps.append(
            dict(wdep=wdep_k, whalf=whalf_k, xtf2=xtf2_k, wpos=wpos_k,
                 xtf=xtf_np, crep=crep_k, xtl=xtl_k,
                 baug=baug_k, caug=caug_k, ident=ident_np)
        )
    return in_maps


def _run(in_maps, trace=False):
    nc = _get_program()
    return run_bass_kernel_spmd(nc, in_maps, list(range(NCORES)), trace=trace)


def _assemble(res):
    out_T = np.concatenate([res.results[k]["out_T"] for k in range(NCORES)], axis=0)
    return np.ascontiguousarray(out_T.T)


def kernel(x, W_pos, b_pos, W_dep, b_dep, edge_token, edge_type):
    in_maps = _prepare_in_maps(x, W_pos, b_pos, W_dep, b_dep, edge_token, edge_type)
    res = _run(in_maps, trace=False)
    return _assemble(res)


def kernel_traced(x, W_pos, b_pos, W_dep, b_dep, edge_token, edge_type):
    """Like kernel() but with NTFF profiling; returns (output, BassKernelResults)."""
    in_maps = _prepare_in_maps(x, W_pos, b_pos, W_dep, b_dep, edge_token, edge_type)
    res = _run(in_maps, trace=True)
    return _assemble(res), res


def install_ntff_shim():
    """The agent image's antenv lacks axon_hooks; recreate it from the boot
    module's ctypes NTFF driver so run_bass_kernel_spmd(trace=True) can
    capture a neuron-profile. Test-only; kernel() never needs this."""
    import sys
    import types

    try:
        from antenv.axon_hooks import get_axon_ntff_profile_hook  # noqa: F401
        return
    except ImportError:
        pass
    from trn_agent_boot.trn_boot import _ntff_profile_via_ctypes

    hook = _ntff_profile_via_ctypes("/opt/axon/libaxon_pjrt.so")
    mod = types.ModuleType("antenv.axon_hooks")
    mod._hook = hook
    mod.get_axon_ntff_profile_hook = lambda: mod._hook
    mod.set_axon_ntff_profile_hook = lambda h: setattr(mod, "_hook", h)
    sys.modules["antenv.axon_hooks"] = mod



# revision 2
# speedup vs baseline: 2.6929x; 2.6929x over previous
"""Trainium2 Bass kernel for the GCNN layer (nn_GCNNLayer_71536975282326).

out = relu( einsum('nd,nde->ne', x, W_pos) + b_pos
            + einsum('nre,nr->ne', einsum('nd,rde->nre', x, W_dep), counts)
            + counts @ b_dep )
with counts[n,r] = #edges (token n, type r).

v2 strategy (8 NeuronCores, SPMD, one program) — the problem is HBM-bound
(242 MiB of f32 weights), so the big lever is weight bytes:

  - All weights are quantized host-side to fp8 e3m4, mean-centered:
    Wq = round_e3m4((W - c) * 32).  The 1/32 is folded into the bf16 moving
    operands; the rank-1 centering correction c*sum_d(x) lands in the bias
    matmul.  End-to-end scale-relative error ~3.3e-3 (gate 2e-2).
    Per-core weight traffic drops 4x: 122 MiB -> ~31 MiB.
  - W_dep is sharded by type (11 full + a half type per core pair, as v1).
  - W_pos is sharded by d-CHUNK (core k takes d rows k*128..k*128+127 of
    every token's matrix).  That makes the self term's PSUM column (= token)
    core-uniform, so self partials accumulate straight into the same
    out_T[e, token] PSUM banks as the dep term and the single ReduceScatter
    sums both.  The v1 AllGather + PE-transpose self machinery is gone.
  - All weight matmuls run with the W 128x128 block as the STATIONARY
    operand (fp8 -> compiler-automatic fast weight load) and a thin bf16
    moving operand: the token's x column (self) or the 150-wide counts*x
    (dep).
  - Weights are pre-tiled on host so each DMA line is 8-10 KiB contiguous:
    one dma_start per ~1 MiB tile = 128 descriptors.
  - Bias + centering corrections: one K=33 f32 matmul per e-chunk.
  - ReduceScatter the [1024,150] f32 partial (core k receives e-chunk k);
    relu; output [128,150]; host concatenates and transposes.
"""

import numpy as np
import ml_dtypes

import concourse.bass as bass
import concourse.tile as tile
from concourse import bacc, mybir
from concourse.bass_utils import run_bass_kernel_spmd

N, D, R = 150, 1024, 92
NCORES = 8
P = 128
DC = D // P            # 8 contraction (d) chunks
EC = D // P            # 8 output (e) chunks
NB = EC // 2           # 4 main psum banks, two e-chunk regions each
NPAD = 256             # region stride inside a psum bank
DEP_SLOTS = 12         # 11 full types + 1 half type per core
DEP_FULL = 11
HC = 4                 # d-chunks in the half slot (types 88..91 split
                       # row-wise across core pairs)
SELF_G = 10            # tokens per self DMA unit
SELF_UNITS = N // SELF_G   # 15
KAUG = 33              # 11 dep rows + 1 half row + 2 centering rows + 19 b_pos rows
QS = 32.0              # fp8 quant scale (power of two: exact in bf16)
F32 = mybir.dt.float32
BF16 = mybir.dt.bfloat16
F8 = mybir.dt.float8e3

NP_BF16 = ml_dtypes.bfloat16
NP_F8 = ml_dtypes.float8_e3m4

_PROG = None


def _build_program():
    nc = bacc.Bacc("TRN2", target_bir_lowering=False, debug=False, num_devices=NCORES)

    # pre-tiled weights: per-partition lines are contiguous in HBM
    wdep = nc.dram_tensor("wdep", [DEP_FULL, P, DC * D], F8, kind="ExternalInput")
    whalf = nc.dram_tensor("whalf", [P, HC * D], F8, kind="ExternalInput")
    wpos = nc.dram_tensor("wpos", [P, N * D], F8, kind="ExternalInput")
    # x^T in tile layout [p, c*N+n] (bf16) + counts/QS replicated across partitions
    xtf = nc.dram_tensor("xtf", [P, DC * N], BF16, kind="ExternalInput")
    xtfh = nc.dram_tensor("xtfh", [P, HC * N], BF16, kind="ExternalInput")
    crep = nc.dram_tensor("crep", [P, DEP_SLOTS * N], BF16, kind="ExternalInput")
    # this core's d-chunk of x^T, prescaled by 1/QS: moving operand of self
    xtl = nc.dram_tensor("xtl", [P, N], BF16, kind="ExternalInput")
    baug = nc.dram_tensor("baug", [KAUG, D], F32, kind="ExternalInput")
    caug = nc.dram_tensor("caug", [KAUG, N], F32, kind="ExternalInput")
    # per-core output: this core's 128-row e-chunk of out_T (host assembles)
    out_T = nc.dram_tensor("out_T", [P, N], F32, kind="ExternalOutput")

    groups = [list(range(NCORES))]

    with tile.TileContext(nc) as tc:
        with (
            tc.tile_pool(name="constp", bufs=1) as constp,
            tc.tile_pool(name="mainps", bufs=1, space=bass.MemorySpace.PSUM) as mainps,
            tc.tile_pool(name="dram", bufs=1, space="DRAM") as dram,
            tc.tile_pool(name="fin", bufs=3) as fin,
        ):
            xtf_t = constp.tile([P, DC * N], BF16)
            nc.scalar.dma_start(out=xtf_t[:], in_=xtf[:])
            crep_t = constp.tile([P, DEP_SLOTS * N], BF16)
            nc.scalar.dma_start(out=crep_t[:], in_=crep[:])
            xtl_t = constp.tile([P, N], BF16)
            nc.gpsimd.dma_start(out=xtl_t[:], in_=xtl[:])
            xtfh_t = constp.tile([P, HC * N], BF16)
            nc.gpsimd.dma_start(out=xtfh_t[:], in_=xtfh[:])
            baug_t = constp.tile([KAUG, D], F32)
            nc.gpsimd.dma_start(out=baug_t[:], in_=baug[:])
            caug_t = constp.tile([KAUG, N], F32)
            nc.sync.dma_start(out=caug_t[:], in_=caug[:])

            accs = [
                mainps.tile([P, 2 * NPAD], F32, name=f"acc{b}", tag=f"acc{b}")
                for b in range(NB)
            ]
            # Bias + centering-correction matmuls first: the single start=True
            # per PSUM bank; the second region's first-touch rides the bank's
            # pending-zero state (v1 pattern).
            for b in range(NB):
                for h in range(2):
                    nc.tensor.matmul(
                        accs[b][:, h * NPAD : h * NPAD + N],
                        baug_t[:, (2 * b + h) * P : (2 * b + h + 1) * P],
                        caug_t[:],
                        start=(h == 0),
                        stop=False,
                    )

            with (
                tc.tile_pool(name="wpool", bufs=6) as wpool,
                tc.tile_pool(name="xspool", bufs=3) as xspool,
            ):
                engs = [nc.sync, nc.gpsimd, nc.scalar]

                def self_unit(g, eng):
                    wt = wpool.tile([P, SELF_G * D], F8, tag="w", name=f"ws{g}")
                    eng.dma_start(
                        out=wt[:], in_=wpos[:, g * SELF_G * D : (g + 1) * SELF_G * D]
                    )
                    for j in range(SELF_G):
                        n = g * SELF_G + j
                        for ec in range(EC):
                            b, h = divmod(ec, 2)
                            nc.tensor.matmul(
                                accs[b][:, h * NPAD + n : h * NPAD + n + 1],
                                wt[:, j * D + ec * P : j * D + (ec + 1) * P],
                                xtl_t[:, n : n + 1],
                                start=False,
                                stop=False,
                            )

                def dep_unit(i, eng):
                    wt = wpool.tile([P, DC * D], F8, tag="w", name=f"wd{i}")
                    eng.dma_start(out=wt[:], in_=wdep[i])
                    xst = xspool.tile([P, DC * N], BF16, tag="xs", name=f"xs{i}")
                    for c in range(DC):
                        nc.vector.tensor_mul(
                            xst[:, c * N : (c + 1) * N],
                            xtf_t[:, c * N : (c + 1) * N],
                            crep_t[:, i * N : (i + 1) * N],
                        )
                    for c in range(DC):
                        for ec in range(EC):
                            b, h = divmod(ec, 2)
                            nc.tensor.matmul(
                                accs[b][:, h * NPAD : h * NPAD + N],
                                wt[:, c * D + ec * P : c * D + (ec + 1) * P],
                                xst[:, c * N : (c + 1) * N],
                                start=False,
                                stop=False,
                            )

                # interleave self (DMA-heavy) and dep (PE-heavy) units
                sched = []
                si = di = 0
                while si < SELF_UNITS or di < DEP_FULL:
                    if di >= DEP_FULL or (si < SELF_UNITS and si * DEP_FULL <= di * SELF_UNITS):
                        sched.append(("s", si)); si += 1
                    else:
                        sched.append(("d", di)); di += 1
                for u, (kind, idx) in enumerate(sched):
                    eng = engs[u % len(engs)]
                    if kind == "s":
                        self_unit(idx, eng)
                    else:
                        dep_unit(idx, eng)

                # half slot: this core's 4 d-chunks of the split type
                wth = wpool.tile([P, HC * D], F8, tag="w", name="whalf")
                nc.sync.dma_start(out=wth[:], in_=whalf[:])
                xsth = xspool.tile([P, HC * N], BF16, tag="xs", name="xsh")
                for c in range(HC):
                    nc.vector.tensor_mul(
                        xsth[:, c * N : (c + 1) * N],
                        xtfh_t[:, c * N : (c + 1) * N],
                        crep_t[:, DEP_FULL * N : (DEP_FULL + 1) * N],
                    )
                for c in range(HC):
                    for ec in range(EC):
                        b, h = divmod(ec, 2)
                        nc.tensor.matmul(
                            accs[b][:, h * NPAD : h * NPAD + N],
                            wth[:, c * D + ec * P : c * D + (ec + 1) * P],
                            xsth[:, c * N : (c + 1) * N],
                            start=False,
                            stop=c == HC - 1 and h == 1,
                        )

            # ---- evacuate + ReduceScatter (core k receives e-chunk k) ----
            ar_main_in = dram.tile([D, N], F32)
            rs_out = dram.tile([P, N], F32)
            for b in range(NB):
                ev = fin.tile([P, 2 * NPAD], F32, tag="ev", name=f"ev{b}")
                nc.vector.tensor_copy(ev[:], accs[b][:])
                for h in range(2):
                    nc.sync.dma_start(
                        out=ar_main_in[(2 * b + h) * P : (2 * b + h + 1) * P, :],
                        in_=ev[:, h * NPAD : h * NPAD + N],
                    )
            nc.gpsimd.collective_compute(
                "ReduceScatter", mybir.AluOpType.add,
                replica_groups=groups, ins=[ar_main_in.opt()], outs=[rs_out.opt()],
            )

            # ---- final: relu(own e-chunk) ----
            mc = fin.tile([P, N], F32, tag="mc")
            nc.gpsimd.dma_start(out=mc[:], in_=rs_out[:])
            oc = fin.tile([P, N], F32, tag="oc")
            nc.vector.tensor_scalar_max(oc[:], mc[:], 0.0)
            nc.sync.dma_start(out=out_T[:], in_=oc[:])

    nc.compile()
    return nc


def _get_program():
    global _PROG
    if _PROG is None:
        _PROG = _build_program()
    return _PROG


def _tile_cpe(w):
    """[(c p), e] f8 -> [p, c*D + e] with contiguous per-partition lines."""
    c = w.shape[0] // P
    return np.ascontiguousarray(
        w.reshape(c, P, w.shape[1]).transpose(1, 0, 2).reshape(P, c * w.shape[1])
    )


def _prepare_in_maps(x, W_pos, b_pos, W_dep, b_dep, edge_token, edge_type):
    x = np.asarray(x, dtype=np.float32)
    W_pos = np.asarray(W_pos, dtype=np.float32)
    b_pos = np.asarray(b_pos, dtype=np.float32)
    W_dep = np.asarray(W_dep, dtype=np.float32)
    b_dep = np.asarray(b_dep, dtype=np.float32)
    edge_token = np.asarray(edge_token)
    edge_type = np.asarray(edge_type)

    counts = np.zeros((N, R), np.float32)
    np.add.at(counts, (edge_token, edge_type), 1.0)

    c_pos = float(W_pos.max() + W_pos.min()) / 2.0
    c_dep = float(W_dep.max() + W_dep.min()) / 2.0
    Wpq = ((W_pos - c_pos) * QS).astype(NP_F8)   # [N, D, D] fp8
    Wdq = ((W_dep - c_dep) * QS).astype(NP_F8)   # [R, D, D] fp8

    xb = x.astype(NP_BF16)
    xbf = xb.astype(np.float32)
    xT16 = np.ascontiguousarray(xb.T)            # [D, N] bf16
    xtf_np = np.ascontiguousarray(
        xT16.reshape(DC, P, N).transpose(1, 0, 2).reshape(P, DC * N)
    )
    sx = xbf.sum(axis=1)                         # [N]

    in_maps = []
    for k in range(NCORES):
        r0 = DEP_FULL * k
        stype = NCORES * DEP_FULL + k // 2   # split type for this core pair
        lower = k % 2 == 0                   # even core: d-chunks 0:4
        c0 = 0 if lower else HC
        t0 = 19 * k
        t1 = min(t0 + 19, N)

        # dep weights: [slot, p, c*D+e]
        wdep_k = np.empty((DEP_FULL, P, DC * D), NP_F8)
        for i in range(DEP_FULL):
            wdep_k[i] = _tile_cpe(Wdq[r0 + i])
        whalf_k = _tile_cpe(Wdq[stype][c0 * P : (c0 + HC) * P])

        # self weights: core k holds d-chunk k of every token: [p, n*D+e]
        wpos_k = np.ascontiguousarray(
            Wpq[:, k * P : (k + 1) * P, :].transpose(1, 0, 2).reshape(P, N * D)
        )

        # self moving operand: x^T chunk k, prescaled by 1/QS (exact in bf16)
        xtl_k = np.ascontiguousarray(
            (x[:, k * P : (k + 1) * P].astype(NP_BF16).astype(np.float32) / QS)
            .astype(NP_BF16).T
        )

        xtfh_k = np.ascontiguousarray(xtf_np[:, c0 * N : (c0 + HC) * N])

        crep_k = np.zeros((1, DEP_SLOTS * N), np.float32)
        crep_k[0, : DEP_FULL * N] = counts[:, r0 : r0 + DEP_FULL].T.reshape(-1) / QS
        crep_k[0, DEP_FULL * N :] = counts[:, stype] / QS
        crep_k = np.ascontiguousarray(
            np.broadcast_to(crep_k.astype(NP_BF16), (P, DEP_SLOTS * N))
        )

        baug_k = np.zeros((KAUG, D), np.float32)
        caug_k = np.zeros((KAUG, N), np.float32)
        baug_k[:DEP_FULL] = b_dep[r0 : r0 + DEP_FULL]
        caug_k[:DEP_FULL] = counts[:, r0 : r0 + DEP_FULL].T
        if lower:
            # split type's bias is counted exactly once, on the even core
            baug_k[DEP_FULL] = b_dep[stype]
            caug_k[DEP_FULL] = counts[:, stype]
        # centering corrections: dep (this core's types) and self (this
        # core's d-chunk); summed across cores by the ReduceScatter
        csum_k = counts[:, r0 : r0 + DEP_FULL].sum(axis=1)
        sxh_k = xbf[:, c0 * P : (c0 + HC) * P].sum(axis=1)
        sxc_k = xbf[:, k * P : (k + 1) * P].sum(axis=1)
        baug_k[DEP_FULL + 1] = c_dep
        caug_k[DEP_FULL + 1] = sx * csum_k + sxh_k * counts[:, stype]
        baug_k[DEP_FULL + 2] = c_pos
        caug_k[DEP_FULL + 2] = sxc_k
        # b_pos rows: one-hot placement of this core's token range
        for j in range(t1 - t0):
            baug_k[DEP_FULL + 3 + j] = b_pos[t0 + j]
            caug_k[DEP_FULL + 3 + j, t0 + j] = 1.0

        in_maps.append(
            dict(wdep=wdep_k, whalf=whalf_k, wpos=wpos_k,
                 xtf=xtf_np, xtfh=xtfh_k, crep=crep_k, xtl=xtl_k,
                 baug=baug_k, caug=caug_k)
        )
    return in_maps


def _run(in_maps, trace=False):
    nc = _get_program()
    return run_bass_kernel_spmd(nc, in_maps, list(range(NCORES)), trace=trace)


def _assemble(res):
    out_T = np.concatenate([res.results[k]["out_T"] for k in range(NCORES)], axis=0)
    return np.ascontiguousarray(out_T.T)


def kernel(x, W_pos, b_pos, W_dep, b_dep, edge_token, edge_type):
    in_maps = _prepare_in_maps(x, W_pos, b_pos, W_dep, b_dep, edge_token, edge_type)
    res = _run(in_maps, trace=False)
    return _assemble(res)


def kernel_traced(x, W_pos, b_pos, W_dep, b_dep, edge_token, edge_type):
    """Like kernel() but with NTFF profiling; returns (output, BassKernelResults)."""
    in_maps = _prepare_in_maps(x, W_pos, b_pos, W_dep, b_dep, edge_token, edge_type)
    res = _run(in_maps, trace=True)
    return _assemble(res), res


def install_ntff_shim():
    """The agent image's antenv lacks axon_hooks; recreate it from the boot
    module's ctypes NTFF driver so run_bass_kernel_spmd(trace=True) can
    capture a neuron-profile. Test-only; kernel() never needs this."""
    import sys
    import types

    try:
        from antenv.axon_hooks import get_axon_ntff_profile_hook  # noqa: F401
        return
    except ImportError:
        pass
    from trn_agent_boot.trn_boot import _ntff_profile_via_ctypes

    hook = _ntff_profile_via_ctypes("/opt/axon/libaxon_pjrt.so")
    mod = types.ModuleType("antenv.axon_hooks")
    mod._hook = hook
    mod.get_axon_ntff_profile_hook = lambda: mod._hook
    mod.set_axon_ntff_profile_hook = lambda h: setattr(mod, "_hook", h)
    sys.modules["antenv.axon_hooks"] = mod


# revision 4
# speedup vs baseline: 3.4400x; 1.2775x over previous
"""Trainium2 Bass kernel for the GCNN layer (nn_GCNNLayer_71536975282326).

out = relu( einsum('nd,nde->ne', x, W_pos) + b_pos
            + einsum('nre,nr->ne', einsum('nd,rde->nre', x, W_dep), counts)
            + counts @ b_dep )
with counts[n,r] = #edges (token n, type r).

v4 strategy (8 NeuronCores, SPMD, one program) — the problem is HBM-bound
(242 MiB of f32 weights), so the levers are weight bytes and overlap:

  - All weights quantized host-side to fp8 e3m4, mean-centered:
    Wq = round_e3m4((W - c) * 32); the 1/32 is folded into the bf16 moving
    operands, the rank-1 centering correction c*sum_d(x) rides the bias
    matmul.  End-to-end scale-relative error ~3.3e-3 (gate 2e-2) and 4x
    less DMA than f32.
  - Sharding is by OUTPUT e-COLUMN: core k holds e-columns 128k..128k+127
    of EVERY W_pos[n] and W_dep[r] (same total bytes as type-sharding) and
    computes its 128 out_T rows COMPLETELY — no ReduceScatter, no
    AllGather, no cross-core synchronization at all.  v2's collective tail
    (44 us) becomes a 3 us DVE+DMA epilogue.
  - Every weight matmul uses the W 128x128 block as the STATIONARY operand
    (fp8 -> compiler-automatic fast weight load, ~25 ns/block) and a thin
    bf16 moving operand: the token's x column (self, width 1, out column =
    token) or the counts-scaled x (dep, width 150).  Everything accumulates
    in ONE PSUM bank [128 e, 150 tokens].
  - counts*x moving operands: one DVE tensor_mul per 4-type group with
    both inputs as stride-0 broadcast views (x repeated over types, counts
    row repeated over d-chunks); the counts row is replicated across
    partitions once by a GpSimd partition_broadcast.
  - Bias: one K=94 f32 matmul (92 counts rows + 2 centering rows), the
    bank's single start=True writer; b_pos lands in the DVE epilogue.
  - Weights pre-tiled on host so every DMA line is 4-10 KiB contiguous:
    one dma_start per 0.5-1.25 MiB unit = 128 descriptors.
"""

import numpy as np
import ml_dtypes

import concourse.bass as bass
import concourse.tile as tile
from concourse import bacc, mybir
from concourse.bass_utils import run_bass_kernel_spmd

N, D, R = 150, 1024, 92
NCORES = 8
P = 128
DC = D // P            # 8 contraction (d) chunks
SELF_G = 10            # tokens per self DMA unit
SELF_UNITS = N // SELF_G   # 15
DEP_G = 4              # types per dep DMA unit
DEP_UNITS = R // DEP_G     # 23
KAUG = R + 2           # 92 counts rows + 2 centering rows
QS = 32.0
F32 = mybir.dt.float32
BF16 = mybir.dt.bfloat16
F8 = mybir.dt.float8e3

NP_BF16 = ml_dtypes.bfloat16
NP_F8 = ml_dtypes.float8_e3m4

_PROG = None


def _build_program():
    nc = bacc.Bacc("TRN2", target_bir_lowering=False, debug=False, num_devices=NCORES)

    # pre-tiled per-core weight slices (e-columns 128k..128k+127):
    # wpos[p, ((n c)) * 128 + e], wdep[p, ((r c)) * 128 + e]
    wpos = nc.dram_tensor("wpos", [P, N * DC * P], F8, kind="ExternalInput")
    wdep = nc.dram_tensor("wdep", [P, R * DC * P], F8, kind="ExternalInput")
    xtf = nc.dram_tensor("xtf", [P, DC * N], BF16, kind="ExternalInput")
    xtl = nc.dram_tensor("xtl", [P, DC * N], BF16, kind="ExternalInput")  # x^T/QS
    crep1 = nc.dram_tensor("crep1", [1, R * N], BF16, kind="ExternalInput")
    baug = nc.dram_tensor("baug", [KAUG, P], F32, kind="ExternalInput")
    caug = nc.dram_tensor("caug", [KAUG, N], F32, kind="ExternalInput")
    bposT = nc.dram_tensor("bposT", [P, N], F32, kind="ExternalInput")
    out_T = nc.dram_tensor("out_T", [P, N], F32, kind="ExternalOutput")

    with tile.TileContext(nc) as tc:
        with (
            tc.tile_pool(name="constp", bufs=1) as constp,
            tc.tile_pool(name="mainps", bufs=1, space=bass.MemorySpace.PSUM) as mainps,
            tc.tile_pool(name="fin", bufs=2) as fin,
        ):
            baug_t = constp.tile([KAUG, P], F32)
            nc.gpsimd.dma_start(out=baug_t[:], in_=baug[:])
            caug_t = constp.tile([KAUG, N], F32)
            nc.gpsimd.dma_start(out=caug_t[:], in_=caug[:])
            xtl_t = constp.tile([P, DC * N], BF16)
            nc.gpsimd.dma_start(out=xtl_t[:], in_=xtl[:])
            xtf_t = constp.tile([P, DC * N], BF16)
            nc.scalar.dma_start(out=xtf_t[:], in_=xtf[:])
            crep1_t = constp.tile([1, R * N], BF16)
            nc.scalar.dma_start(out=crep1_t[:], in_=crep1[:])
            bposT_t = constp.tile([P, N], F32)
            nc.scalar.dma_start(out=bposT_t[:], in_=bposT[:])
            crep_t = constp.tile([P, R * N], BF16)
            nc.gpsimd.partition_broadcast(crep_t[:], crep1_t[:])

            acc = mainps.tile([P, 512], F32, name="acc", tag="acc")
            # bias + centering corrections: the bank's single start=True writer
            nc.tensor.matmul(
                acc[:, 0:N], baug_t[:], caug_t[:], start=True, stop=False,
            )

            with (
                tc.tile_pool(name="wpool", bufs=8) as wpool,
                tc.tile_pool(name="xspool", bufs=3) as xspool,
            ):
                engs = [nc.sync, nc.gpsimd, nc.scalar]

                def self_unit(g, eng, stop_last):
                    wt = wpool.tile([P, SELF_G * DC * P], F8, tag="w", name=f"ws{g}")
                    eng.dma_start(
                        out=wt[:],
                        in_=wpos[:, g * SELF_G * DC * P : (g + 1) * SELF_G * DC * P],
                    )
                    for j in range(SELF_G):
                        n = g * SELF_G + j
                        for c in range(DC):
                            nc.tensor.matmul(
                                acc[:, n : n + 1],
                                wt[:, (j * DC + c) * P : (j * DC + c + 1) * P],
                                xtl_t[:, c * N + n : c * N + n + 1],
                                start=False,
                                stop=stop_last and j == SELF_G - 1 and c == DC - 1,
                            )

                def dep_unit(g, eng, stop_last):
                    wt = wpool.tile([P, DEP_G * DC * P], F8, tag="w", name=f"wd{g}")
                    eng.dma_start(
                        out=wt[:],
                        in_=wdep[:, g * DEP_G * DC * P : (g + 1) * DEP_G * DC * P],
                    )
                    # xs[(tr c n)] = x^T[(c n)] * counts[type]/QS, both broadcast
                    xst = xspool.tile([P, DEP_G * DC * N], BF16, tag="xs", name=f"xs{g}")
                    nc.vector.tensor_mul(
                        xst[:].rearrange("p (t c n) -> p t c n", t=DEP_G, c=DC),
                        xtf_t[:, None, :]
                        .rearrange("p t (c n) -> p t c n", c=DC)
                        .broadcast_to([P, DEP_G, DC, N]),
                        crep_t[:, g * DEP_G * N : (g + 1) * DEP_G * N, None]
                        .rearrange("p (t n) c -> p t c n", t=DEP_G)
                        .broadcast_to([P, DEP_G, DC, N]),
                    )
                    for t in range(DEP_G):
                        for c in range(DC):
                            nc.tensor.matmul(
                                acc[:, 0:N],
                                wt[:, (t * DC + c) * P : (t * DC + c + 1) * P],
                                xst[:, (t * DC + c) * N : (t * DC + c + 1) * N],
                                start=False,
                                stop=stop_last and t == DEP_G - 1 and c == DC - 1,
                            )

                # interleave self (DMA-heavy) and dep (PE-heavy) units;
                # force a dep unit last so the stop lands on a full-width MM
                sched = []
                si = di = 0
                while si < SELF_UNITS or di < DEP_UNITS - 1:
                    if di >= DEP_UNITS - 1 or (
                        si < SELF_UNITS and si * (DEP_UNITS - 1) <= di * SELF_UNITS
                    ):
                        sched.append(("s", si)); si += 1
                    else:
                        sched.append(("d", di)); di += 1
                sched.append(("d", DEP_UNITS - 1))

                for u, (kind, idx) in enumerate(sched):
                    eng = engs[u % len(engs)]
                    last = u == len(sched) - 1
                    if kind == "s":
                        self_unit(idx, eng, last)
                    else:
                        dep_unit(idx, eng, last)

            # ---- epilogue: out = relu(acc + b_pos^T), straight to HBM ----
            oc = fin.tile([P, N], F32, tag="oc")
            nc.vector.scalar_tensor_tensor(
                oc[:], acc[:, 0:N], 0.0, bposT_t[:],
                mybir.AluOpType.add, mybir.AluOpType.add,
            )
            nc.vector.tensor_scalar_max(oc[:], oc[:], 0.0)
            nc.sync.dma_start(out=out_T[:], in_=oc[:])

    nc.compile()
    return nc


def _get_program():
    global _PROG
    if _PROG is None:
        _PROG = _build_program()
    return _PROG


def _prepare_in_maps(x, W_pos, b_pos, W_dep, b_dep, edge_token, edge_type):
    x = np.asarray(x, dtype=np.float32)
    W_pos = np.asarray(W_pos, dtype=np.float32)
    b_pos = np.asarray(b_pos, dtype=np.float32)
    W_dep = np.asarray(W_dep, dtype=np.float32)
    b_dep = np.asarray(b_dep, dtype=np.float32)
    edge_token = np.asarray(edge_token)
    edge_type = np.asarray(edge_type)

    counts = np.zeros((N, R), np.float32)
    np.add.at(counts, (edge_token, edge_type), 1.0)

    c_pos = float(W_pos.max() + W_pos.min()) / 2.0
    c_dep = float(W_dep.max() + W_dep.min()) / 2.0
    Wpq = ((W_pos - c_pos) * QS).astype(NP_F8)   # [N, D, D] fp8
    Wdq = ((W_dep - c_dep) * QS).astype(NP_F8)   # [R, D, D] fp8

    xb = x.astype(NP_BF16)
    xbf = xb.astype(np.float32)
    xT16 = np.ascontiguousarray(xb.T)
    xtf_np = np.ascontiguousarray(
        xT16.reshape(DC, P, N).transpose(1, 0, 2).reshape(P, DC * N)
    )
    xtl_np = np.ascontiguousarray(
        (xbf.T / QS).astype(NP_BF16).reshape(DC, P, N).transpose(1, 0, 2)
        .reshape(P, DC * N)
    )
    sx = xbf.sum(axis=1)
    csum = counts.sum(axis=1)
    crep1_np = np.ascontiguousarray((counts.T / QS).astype(NP_BF16).reshape(1, R * N))

    in_maps = []
    for k in range(NCORES):
        sl = slice(k * P, (k + 1) * P)
        wpos_k = np.ascontiguousarray(
            Wpq[:, :, sl].reshape(N, DC, P, P).transpose(2, 0, 1, 3)
            .reshape(P, N * DC * P)
        )
        wdep_k = np.ascontiguousarray(
            Wdq[:, :, sl].reshape(R, DC, P, P).transpose(2, 0, 1, 3)
            .reshape(P, R * DC * P)
        )
        baug_k = np.empty((KAUG, P), np.float32)
        baug_k[:R] = b_dep[:, sl]
        baug_k[R] = c_dep
        baug_k[R + 1] = c_pos
        caug_k = np.empty((KAUG, N), np.float32)
        caug_k[:R] = counts.T
        caug_k[R] = sx * csum
        caug_k[R + 1] = sx
        bposT_k = np.ascontiguousarray(b_pos[:, sl].T)

        in_maps.append(
            dict(wpos=wpos_k, wdep=wdep_k, xtf=xtf_np, xtl=xtl_np,
                 crep1=crep1_np, baug=baug_k, caug=caug_k, bposT=bposT_k)
        )
    return in_maps


def _run(in_maps, trace=False):
    nc = _get_program()
    return run_bass_kernel_spmd(nc, in_maps, list(range(NCORES)), trace=trace)


def _assemble(res):
    out_T = np.concatenate([res.results[k]["out_T"] for k in range(NCORES)], axis=0)
    return np.ascontiguousarray(out_T.T)


def kernel(x, W_pos, b_pos, W_dep, b_dep, edge_token, edge_type):
    in_maps = _prepare_in_maps(x, W_pos, b_pos, W_dep, b_dep, edge_token, edge_type)
    res = _run(in_maps, trace=False)
    return _assemble(res)


def kernel_traced(x, W_pos, b_pos, W_dep, b_dep, edge_token, edge_type):
    """Like kernel() but with NTFF profiling; returns (output, BassKernelResults)."""
    in_maps = _prepare_in_maps(x, W_pos, b_pos, W_dep, b_dep, edge_token, edge_type)
    res = _run(in_maps, trace=True)
    return _assemble(res), res


def install_ntff_shim():
    """The agent image's antenv lacks axon_hooks; recreate it from the boot
    module's ctypes NTFF driver so run_bass_kernel_spmd(trace=True) can
    capture a neuron-profile. Test-only; kernel() never needs this."""
    import sys
    import types

    try:
        from antenv.axon_hooks import get_axon_ntff_profile_hook  # noqa: F401
        return
    except ImportError:
        pass
    from trn_agent_boot.trn_boot import _ntff_profile_via_ctypes

    hook = _ntff_profile_via_ctypes("/opt/axon/libaxon_pjrt.so")
    mod = types.ModuleType("antenv.axon_hooks")
    mod._hook = hook
    mod.get_axon_ntff_profile_hook = lambda: mod._hook
    mod.set_axon_ntff_profile_hook = lambda h: setattr(mod, "_hook", h)
    sys.modules["antenv.axon_hooks"] = mod


# revision 7
# speedup vs baseline: 4.0683x; 1.1826x over previous
"""Trainium2 Bass kernel for the GCNN layer (nn_GCNNLayer_71536975282326).

out = relu( einsum('nd,nde->ne', x, W_pos) + b_pos
            + einsum('nre,nr->ne', einsum('nd,rde->nre', x, W_dep), counts)
            + counts @ b_dep )
with counts[n,r] = #edges (token n, type r).

v4 strategy (8 NeuronCores, SPMD, one program) — the problem is HBM-bound
(242 MiB of f32 weights), so the levers are weight bytes and overlap:

  - All weights quantized host-side to fp8 e3m4, mean-centered:
    Wq = round_e3m4((W - c) * 32); the 1/32 is folded into the bf16 moving
    operands, the rank-1 centering correction c*sum_d(x) rides the bias
    matmul.  End-to-end scale-relative error ~3.3e-3 (gate 2e-2) and 4x
    less DMA than f32.
  - Sharding is by OUTPUT e-COLUMN: core k holds e-columns 128k..128k+127
    of EVERY W_pos[n] and W_dep[r] (same total bytes as type-sharding) and
    computes its 128 out_T rows COMPLETELY — no ReduceScatter, no
    AllGather, no cross-core synchronization at all.  v2's collective tail
    (44 us) becomes a 3 us DVE+DMA epilogue.
  - Every weight matmul uses the W 128x128 block as the STATIONARY operand
    (fp8 -> compiler-automatic fast weight load, ~25 ns/block) and a thin
    bf16 moving operand: the token's x column (self, width 1, out column =
    token) or the counts-scaled x (dep, width 150).  Everything accumulates
    in ONE PSUM bank [128 e, 150 tokens].
  - counts*x moving operands: one DVE tensor_mul per 4-type group with
    both inputs as stride-0 broadcast views (x repeated over types, counts
    row repeated over d-chunks); the counts row is replicated across
    partitions once by a GpSimd partition_broadcast.
  - Bias: one K=94 f32 matmul (92 counts rows + 2 centering rows), the
    bank's single start=True writer; b_pos lands in the DVE epilogue.
  - Weights pre-tiled on host so every DMA line is 4-10 KiB contiguous:
    one dma_start per 0.5-1.25 MiB unit = 128 descriptors.
"""

import numpy as np
import ml_dtypes

import concourse.bass as bass
import concourse.tile as tile
from concourse import bacc, mybir
from concourse.bass_utils import run_bass_kernel_spmd

N, D, R = 150, 1024, 92
NCORES = 8
P = 128
DC = D // P            # 8 contraction (d) chunks
SELF_G = 10            # tokens per self DMA unit
SELF_UNITS = N // SELF_G   # 15
DEP_G = 4              # types per dep DMA unit
DEP_UNITS = R // DEP_G     # 23
KAUG = R + 2           # 92 counts rows + 2 centering rows
QS = 32.0
F32 = mybir.dt.float32
BF16 = mybir.dt.bfloat16
F8 = mybir.dt.float8e3

NP_BF16 = ml_dtypes.bfloat16
NP_F8 = ml_dtypes.float8_e3m4

_PROG = None


def _build_program():
    nc = bacc.Bacc("TRN2", target_bir_lowering=False, debug=False, num_devices=NCORES)

    # pre-tiled per-core weight slices (e-columns 128k..128k+127):
    # wpos[p, ((n c)) * 128 + e], wdep[p, ((r c)) * 128 + e]
    wpos = nc.dram_tensor("wpos", [P, N * DC * P], F8, kind="ExternalInput")
    wdep = nc.dram_tensor("wdep", [P, R * DC * P], F8, kind="ExternalInput")
    xtf = nc.dram_tensor("xtf", [P, DC * N], BF16, kind="ExternalInput")
    xtl = nc.dram_tensor("xtl", [P, DC * N], BF16, kind="ExternalInput")  # x^T/QS
    crep1 = nc.dram_tensor("crep1", [1, R * N], BF16, kind="ExternalInput")
    baug = nc.dram_tensor("baug", [KAUG, P], F32, kind="ExternalInput")
    caug = nc.dram_tensor("caug", [KAUG, N], F32, kind="ExternalInput")
    bposT = nc.dram_tensor("bposT", [P, N], F32, kind="ExternalInput")
    out_T = nc.dram_tensor("out_T", [P, N], F32, kind="ExternalOutput")

    with tile.TileContext(nc) as tc:
        with (
            tc.tile_pool(name="constp", bufs=1) as constp,
            tc.tile_pool(name="mainps", bufs=1, space=bass.MemorySpace.PSUM) as mainps,
            tc.tile_pool(name="fin", bufs=2) as fin,
        ):
            baug_t = constp.tile([KAUG, P], F32)
            nc.sync.dma_start(out=baug_t[:], in_=baug[:])
            caug_t = constp.tile([KAUG, N], F32)
            nc.sync.dma_start(out=caug_t[:], in_=caug[:])
            crep1_t = constp.tile([1, R * N], BF16)
            nc.scalar.dma_start(out=crep1_t[:], in_=crep1[:])
            xtf_t = constp.tile([P, DC * N], BF16)
            nc.scalar.dma_start(out=xtf_t[:], in_=xtf[:])
            bposT_t = constp.tile([P, N], F32)
            nc.scalar.dma_start(out=bposT_t[:], in_=bposT[:])
            # x^T/QS for the self moving operand: derived on DVE, saves a load
            xtl_t = constp.tile([P, DC * N], BF16)
            nc.vector.tensor_scalar_mul(xtl_t[:], xtf_t[:], 1.0 / QS)
            # counts/QS replicated across partitions in per-group chunks so
            # the first dep unit's xs only waits ~1us, not a 20us monolith
            crep_t = constp.tile([P, R * N], BF16)
            for g in range(DEP_UNITS):
                nc.gpsimd.partition_broadcast(
                    crep_t[:, g * DEP_G * N : (g + 1) * DEP_G * N],
                    crep1_t[:, g * DEP_G * N : (g + 1) * DEP_G * N],
                )

            acc = mainps.tile([P, 512], F32, name="acc", tag="acc")
            # bias + centering corrections: the bank's single start=True writer
            nc.tensor.matmul(
                acc[:, 0:N], baug_t[:], caug_t[:], start=True, stop=False,
            )

            with (
                tc.tile_pool(name="wpool", bufs=8) as wpool,
                tc.tile_pool(name="xspool", bufs=4) as xspool,
            ):
                # gpsimd is reserved for the partition_broadcast chain; the
                # W stream triggers rotate over the two HWDGE queues
                engs = [nc.sync, nc.scalar]

                def self_unit(g, eng, stop_last):
                    wt = wpool.tile([P, SELF_G * DC * P], F8, tag="w", name=f"ws{g}")
                    eng.dma_start(
                        out=wt[:],
                        in_=wpos[:, g * SELF_G * DC * P : (g + 1) * SELF_G * DC * P],
                    )
                    for j in range(SELF_G):
                        n = g * SELF_G + j
                        for c in range(DC):
                            nc.tensor.matmul(
                                acc[:, n : n + 1],
                                wt[:, (j * DC + c) * P : (j * DC + c + 1) * P],
                                xtl_t[:, c * N + n : c * N + n + 1],
                                start=False,
                                stop=stop_last and j == SELF_G - 1 and c == DC - 1,
                            )

                def dep_unit(g, eng, stop_last):
                    wt = wpool.tile([P, DEP_G * DC * P], F8, tag="w", name=f"wd{g}")
                    eng.dma_start(
                        out=wt[:],
                        in_=wdep[:, g * DEP_G * DC * P : (g + 1) * DEP_G * DC * P],
                    )
                    # xs[(tr c n)] = x^T[(c n)] * counts[type]/QS, both broadcast
                    xst = xspool.tile([P, DEP_G * DC * N], BF16, tag="xs", name=f"xs{g}")
                    nc.vector.tensor_mul(
                        xst[:].rearrange("p (t c n) -> p t c n", t=DEP_G, c=DC),
                        xtf_t[:, None, :]
                        .rearrange("p t (c n) -> p t c n", c=DC)
                        .broadcast_to([P, DEP_G, DC, N]),
                        crep_t[:, g * DEP_G * N : (g + 1) * DEP_G * N, None]
                        .rearrange("p (t n) c -> p t c n", t=DEP_G)
                        .broadcast_to([P, DEP_G, DC, N]),
                    )
                    for t in range(DEP_G):
                        for c in range(DC):
                            nc.tensor.matmul(
                                acc[:, 0:N],
                                wt[:, (t * DC + c) * P : (t * DC + c + 1) * P],
                                xst[:, (t * DC + c) * N : (t * DC + c + 1) * N],
                                start=False,
                                stop=stop_last and t == DEP_G - 1 and c == DC - 1,
                            )

                # interleave self (DMA-heavy) and dep (PE-heavy) units;
                # force a dep unit last so the stop lands on a full-width MM
                sched = []
                si = di = 0
                while si < SELF_UNITS or di < DEP_UNITS - 1:
                    if di >= DEP_UNITS - 1 or (
                        si < SELF_UNITS and si * (DEP_UNITS - 1) <= di * SELF_UNITS
                    ):
                        sched.append(("s", si)); si += 1
                    else:
                        sched.append(("d", di)); di += 1
                sched.append(("d", DEP_UNITS - 1))

                for u, (kind, idx) in enumerate(sched):
                    eng = engs[u % len(engs)]
                    last = u == len(sched) - 1
                    if kind == "s":
                        self_unit(idx, eng, last)
                    else:
                        dep_unit(idx, eng, last)

            # ---- epilogue: out = relu(acc + b_pos^T), straight to HBM ----
            oc = fin.tile([P, N], F32, tag="oc")
            nc.vector.scalar_tensor_tensor(
                oc[:], acc[:, 0:N], 0.0, bposT_t[:],
                mybir.AluOpType.add, mybir.AluOpType.add,
            )
            nc.vector.tensor_scalar_max(oc[:], oc[:], 0.0)
            nc.sync.dma_start(out=out_T[:], in_=oc[:])

    nc.compile()
    return nc


def _get_program():
    global _PROG
    if _PROG is None:
        _PROG = _build_program()
    return _PROG


def _prepare_in_maps(x, W_pos, b_pos, W_dep, b_dep, edge_token, edge_type):
    x = np.asarray(x, dtype=np.float32)
    W_pos = np.asarray(W_pos, dtype=np.float32)
    b_pos = np.asarray(b_pos, dtype=np.float32)
    W_dep = np.asarray(W_dep, dtype=np.float32)
    b_dep = np.asarray(b_dep, dtype=np.float32)
    edge_token = np.asarray(edge_token)
    edge_type = np.asarray(edge_type)

    counts = np.zeros((N, R), np.float32)
    np.add.at(counts, (edge_token, edge_type), 1.0)

    c_pos = float(W_pos.max() + W_pos.min()) / 2.0
    c_dep = float(W_dep.max() + W_dep.min()) / 2.0
    Wpq = ((W_pos - c_pos) * QS).astype(NP_F8)   # [N, D, D] fp8
    Wdq = ((W_dep - c_dep) * QS).astype(NP_F8)   # [R, D, D] fp8

    xb = x.astype(NP_BF16)
    xbf = xb.astype(np.float32)
    xT16 = np.ascontiguousarray(xb.T)
    xtf_np = np.ascontiguousarray(
        xT16.reshape(DC, P, N).transpose(1, 0, 2).reshape(P, DC * N)
    )
    xtl_np = np.ascontiguousarray(
        (xbf.T / QS).astype(NP_BF16).reshape(DC, P, N).transpose(1, 0, 2)
        .reshape(P, DC * N)
    )
    sx = xbf.sum(axis=1)
    csum = counts.sum(axis=1)
    crep1_np = np.ascontiguousarray((counts.T / QS).astype(NP_BF16).reshape(1, R * N))

    in_maps = []
    for k in range(NCORES):
        sl = slice(k * P, (k + 1) * P)
        wpos_k = np.ascontiguousarray(
            Wpq[:, :, sl].reshape(N, DC, P, P).transpose(2, 0, 1, 3)
            .reshape(P, N * DC * P)
        )
        wdep_k = np.ascontiguousarray(
            Wdq[:, :, sl].reshape(R, DC, P, P).transpose(2, 0, 1, 3)
            .reshape(P, R * DC * P)
        )
        baug_k = np.empty((KAUG, P), np.float32)
        baug_k[:R] = b_dep[:, sl]
        baug_k[R] = c_dep
        baug_k[R + 1] = c_pos
        caug_k = np.empty((KAUG, N), np.float32)
        caug_k[:R] = counts.T
        caug_k[R] = sx * csum
        caug_k[R + 1] = sx
        bposT_k = np.ascontiguousarray(b_pos[:, sl].T)

        in_maps.append(
            dict(wpos=wpos_k, wdep=wdep_k, xtf=xtf_np, xtl=xtl_np,
                 crep1=crep1_np, baug=baug_k, caug=caug_k, bposT=bposT_k)
        )
    return in_maps


def _run(in_maps, trace=False):
    nc = _get_program()
    return run_bass_kernel_spmd(nc, in_maps, list(range(NCORES)), trace=trace)


def _assemble(res):
    out_T = np.concatenate([res.results[k]["out_T"] for k in range(NCORES)], axis=0)
    return np.ascontiguousarray(out_T.T)


def kernel(x, W_pos, b_pos, W_dep, b_dep, edge_token, edge_type):
    in_maps = _prepare_in_maps(x, W_pos, b_pos, W_dep, b_dep, edge_token, edge_type)
    res = _run(in_maps, trace=False)
    return _assemble(res)


def kernel_traced(x, W_pos, b_pos, W_dep, b_dep, edge_token, edge_type):
    """Like kernel() but with NTFF profiling; returns (output, BassKernelResults)."""
    in_maps = _prepare_in_maps(x, W_pos, b_pos, W_dep, b_dep, edge_token, edge_type)
    res = _run(in_maps, trace=True)
    return _assemble(res), res


def install_ntff_shim():
    """The agent image's antenv lacks axon_hooks; recreate it from the boot
    module's ctypes NTFF driver so run_bass_kernel_spmd(trace=True) can
    capture a neuron-profile. Test-only; kernel() never needs this."""
    import sys
    import types

    try:
        from antenv.axon_hooks import get_axon_ntff_profile_hook  # noqa: F401
        return
    except ImportError:
        pass
    from trn_agent_boot.trn_boot import _ntff_profile_via_ctypes

    hook = _ntff_profile_via_ctypes("/opt/axon/libaxon_pjrt.so")
    mod = types.ModuleType("antenv.axon_hooks")
    mod._hook = hook
    mod.get_axon_ntff_profile_hook = lambda: mod._hook
    mod.set_axon_ntff_profile_hook = lambda h: setattr(mod, "_hook", h)
    sys.modules["antenv.axon_hooks"] = mod


# revision 9
# speedup vs baseline: 4.0717x; 1.0008x over previous
"""Trainium2 Bass kernel for the GCNN layer (nn_GCNNLayer_71536975282326).

out = relu( einsum('nd,nde->ne', x, W_pos) + b_pos
            + einsum('nre,nr->ne', einsum('nd,rde->nre', x, W_dep), counts)
            + counts @ b_dep )
with counts[n,r] = #edges (token n, type r).

v4 strategy (8 NeuronCores, SPMD, one program) — the problem is HBM-bound
(242 MiB of f32 weights), so the levers are weight bytes and overlap:

  - All weights quantized host-side to fp8 e3m4, mean-centered:
    Wq = round_e3m4((W - c) * 32); the 1/32 is folded into the bf16 moving
    operands, the rank-1 centering correction c*sum_d(x) rides the bias
    matmul.  End-to-end scale-relative error ~3.3e-3 (gate 2e-2) and 4x
    less DMA than f32.
  - Sharding is by OUTPUT e-COLUMN: core k holds e-columns 128k..128k+127
    of EVERY W_pos[n] and W_dep[r] (same total bytes as type-sharding) and
    computes its 128 out_T rows COMPLETELY — no ReduceScatter, no
    AllGather, no cross-core synchronization at all.  v2's collective tail
    (44 us) becomes a 3 us DVE+DMA epilogue.
  - Every weight matmul uses the W 128x128 block as the STATIONARY operand
    (fp8 -> compiler-automatic fast weight load, ~25 ns/block) and a thin
    bf16 moving operand: the token's x column (self, width 1, out column =
    token) or the counts-scaled x (dep, width 150).  Everything accumulates
    in ONE PSUM bank [128 e, 150 tokens].
  - counts*x moving operands: one DVE tensor_mul per 4-type group with
    both inputs as stride-0 broadcast views (x repeated over types, counts
    row repeated over d-chunks); the counts row is replicated across
    partitions once by a GpSimd partition_broadcast.
  - Bias: one K=94 f32 matmul (92 counts rows + 2 centering rows), the
    bank's single start=True writer; b_pos lands in the DVE epilogue.
  - Weights pre-tiled on host so every DMA line is 4-10 KiB contiguous:
    one dma_start per 0.5-1.25 MiB unit = 128 descriptors.
"""

import numpy as np
import ml_dtypes

import concourse.bass as bass
import concourse.tile as tile
from concourse import bacc, mybir
from concourse.bass_utils import run_bass_kernel_spmd

N, D, R = 150, 1024, 92
NCORES = 8
P = 128
DC = D // P            # 8 contraction (d) chunks
SELF_G = 10            # tokens per self DMA unit
SELF_UNITS = N // SELF_G   # 15
DEP_G = 4              # types per dep DMA unit
DEP_UNITS = R // DEP_G     # 23
KAUG = R + 2           # 92 counts rows + 2 centering rows
QS = 32.0
F32 = mybir.dt.float32
BF16 = mybir.dt.bfloat16
F8 = mybir.dt.float8e3

NP_BF16 = ml_dtypes.bfloat16
NP_F8 = ml_dtypes.float8_e3m4

_PROG = None


def _build_program():
    nc = bacc.Bacc("TRN2", target_bir_lowering=False, debug=False, num_devices=NCORES)

    # pre-tiled per-core weight slices (e-columns 128k..128k+127):
    # wpos[p, ((n c)) * 128 + e], wdep[p, ((r c)) * 128 + e]
    wpos = nc.dram_tensor("wpos", [P, N * DC * P], F8, kind="ExternalInput")
    wdep = nc.dram_tensor("wdep", [P, R * DC * P], F8, kind="ExternalInput")
    xtf = nc.dram_tensor("xtf", [P, DC * N], BF16, kind="ExternalInput")
    xtl = nc.dram_tensor("xtl", [P, DC * N], BF16, kind="ExternalInput")  # x^T/QS
    crep1 = nc.dram_tensor("crep1", [1, R * N], BF16, kind="ExternalInput")
    baug = nc.dram_tensor("baug", [KAUG, P], F32, kind="ExternalInput")
    caug = nc.dram_tensor("caug", [KAUG, N], F32, kind="ExternalInput")
    bposT = nc.dram_tensor("bposT", [P, N], F32, kind="ExternalInput")
    out_T = nc.dram_tensor("out_T", [P, N], F32, kind="ExternalOutput")

    with tile.TileContext(nc) as tc:
        with (
            tc.tile_pool(name="constp", bufs=1) as constp,
            tc.tile_pool(name="mainps", bufs=1, space=bass.MemorySpace.PSUM) as mainps,
            tc.tile_pool(name="fin", bufs=2) as fin,
        ):
            baug_t = constp.tile([KAUG, P], F32)
            nc.sync.dma_start(out=baug_t[:], in_=baug[:])
            caug_t = constp.tile([KAUG, N], F32)
            nc.sync.dma_start(out=caug_t[:], in_=caug[:])
            crep1_t = constp.tile([1, R * N], BF16)
            nc.scalar.dma_start(out=crep1_t[:], in_=crep1[:])
            xtf_t = constp.tile([P, DC * N], BF16)
            nc.scalar.dma_start(out=xtf_t[:], in_=xtf[:])
            bposT_t = constp.tile([P, N], F32)
            nc.scalar.dma_start(out=bposT_t[:], in_=bposT[:])
            # x^T/QS for the self moving operand: derived on DVE, saves a load
            xtl_t = constp.tile([P, DC * N], BF16)
            nc.vector.tensor_scalar_mul(xtl_t[:], xtf_t[:], 1.0 / QS)
            # counts/QS replicated across partitions in per-group chunks so
            # the first dep unit's xs only waits ~1us, not a 20us monolith
            crep_t = constp.tile([P, R * N], BF16)
            for g in range(DEP_UNITS):
                nc.gpsimd.partition_broadcast(
                    crep_t[:, g * DEP_G * N : (g + 1) * DEP_G * N],
                    crep1_t[:, g * DEP_G * N : (g + 1) * DEP_G * N],
                )

            acc = mainps.tile([P, 512], F32, name="acc", tag="acc")
            # bias + centering corrections: the bank's single start=True writer
            nc.tensor.matmul(
                acc[:, 0:N], baug_t[:], caug_t[:], start=True, stop=False,
            )

            with (
                tc.tile_pool(name="wpool", bufs=9) as wpool,
                tc.tile_pool(name="xspool", bufs=5) as xspool,
            ):
                # gpsimd is reserved for the partition_broadcast chain; the
                # W stream triggers rotate over the two HWDGE queues
                engs = [nc.sync, nc.scalar]

                def self_unit(g, eng, stop_last):
                    wt = wpool.tile([P, SELF_G * DC * P], F8, tag="w", name=f"ws{g}")
                    eng.dma_start(
                        out=wt[:],
                        in_=wpos[:, g * SELF_G * DC * P : (g + 1) * SELF_G * DC * P],
                    )
                    for j in range(SELF_G):
                        n = g * SELF_G + j
                        for c in range(DC):
                            nc.tensor.matmul(
                                acc[:, n : n + 1],
                                wt[:, (j * DC + c) * P : (j * DC + c + 1) * P],
                                xtl_t[:, c * N + n : c * N + n + 1],
                                start=False,
                                stop=stop_last and j == SELF_G - 1 and c == DC - 1,
                            )

                def dep_unit(g, eng, stop_last):
                    wt = wpool.tile([P, DEP_G * DC * P], F8, tag="w", name=f"wd{g}")
                    eng.dma_start(
                        out=wt[:],
                        in_=wdep[:, g * DEP_G * DC * P : (g + 1) * DEP_G * DC * P],
                    )
                    # xs[(tr c n)] = x^T[(c n)] * counts[type]/QS, both broadcast
                    xst = xspool.tile([P, DEP_G * DC * N], BF16, tag="xs", name=f"xs{g}")
                    nc.vector.tensor_mul(
                        xst[:].rearrange("p (t c n) -> p t c n", t=DEP_G, c=DC),
                        xtf_t[:, None, :]
                        .rearrange("p t (c n) -> p t c n", c=DC)
                        .broadcast_to([P, DEP_G, DC, N]),
                        crep_t[:, g * DEP_G * N : (g + 1) * DEP_G * N, None]
                        .rearrange("p (t n) c -> p t c n", t=DEP_G)
                        .broadcast_to([P, DEP_G, DC, N]),
                    )
                    for t in range(DEP_G):
                        for c in range(DC):
                            nc.tensor.matmul(
                                acc[:, 0:N],
                                wt[:, (t * DC + c) * P : (t * DC + c + 1) * P],
                                xst[:, (t * DC + c) * N : (t * DC + c + 1) * N],
                                start=False,
                                stop=stop_last and t == DEP_G - 1 and c == DC - 1,
                            )

                # interleave self (DMA-heavy) and dep (PE+DVE-heavy) units;
                # two self units first (gives the DVE xs pipeline lead time),
                # and a dep unit last so the stop lands on a full-width MM
                sched = [("s", 0), ("s", 1)]
                si, di = 2, 0
                while si < SELF_UNITS or di < DEP_UNITS - 1:
                    if di >= DEP_UNITS - 1 or (
                        si < SELF_UNITS
                        and (si - 2) * (DEP_UNITS - 1) <= di * (SELF_UNITS - 2)
                    ):
                        sched.append(("s", si)); si += 1
                    else:
                        sched.append(("d", di)); di += 1
                sched.append(("d", DEP_UNITS - 1))

                for u, (kind, idx) in enumerate(sched):
                    eng = engs[u % len(engs)]
                    last = u == len(sched) - 1
                    if kind == "s":
                        self_unit(idx, eng, last)
                    else:
                        dep_unit(idx, eng, last)

            # ---- epilogue: out = relu(acc + b_pos^T), straight to HBM ----
            oc = fin.tile([P, N], F32, tag="oc")
            nc.vector.scalar_tensor_tensor(
                oc[:], acc[:, 0:N], 0.0, bposT_t[:],
                mybir.AluOpType.add, mybir.AluOpType.add,
            )
            nc.vector.tensor_scalar_max(oc[:], oc[:], 0.0)
            nc.sync.dma_start(out=out_T[:], in_=oc[:])

    nc.compile()
    return nc


def _get_program():
    global _PROG
    if _PROG is None:
        _PROG = _build_program()
    return _PROG


def _prepare_in_maps(x, W_pos, b_pos, W_dep, b_dep, edge_token, edge_type):
    x = np.asarray(x, dtype=np.float32)
    W_pos = np.asarray(W_pos, dtype=np.float32)
    b_pos = np.asarray(b_pos, dtype=np.float32)
    W_dep = np.asarray(W_dep, dtype=np.float32)
    b_dep = np.asarray(b_dep, dtype=np.float32)
    edge_token = np.asarray(edge_token)
    edge_type = np.asarray(edge_type)

    counts = np.zeros((N, R), np.float32)
    np.add.at(counts, (edge_token, edge_type), 1.0)

    c_pos = float(W_pos.max() + W_pos.min()) / 2.0
    c_dep = float(W_dep.max() + W_dep.min()) / 2.0
    Wpq = ((W_pos - c_pos) * QS).astype(NP_F8)   # [N, D, D] fp8
    Wdq = ((W_dep - c_dep) * QS).astype(NP_F8)   # [R, D, D] fp8

    xb = x.astype(NP_BF16)
    xbf = xb.astype(np.float32)
    xT16 = np.ascontiguousarray(xb.T)
    xtf_np = np.ascontiguousarray(
        xT16.reshape(DC, P, N).transpose(1, 0, 2).reshape(P, DC * N)
    )
    xtl_np = np.ascontiguousarray(
        (xbf.T / QS).astype(NP_BF16).reshape(DC, P, N).transpose(1, 0, 2)
        .reshape(P, DC * N)
    )
    sx = xbf.sum(axis=1)
    csum = counts.sum(axis=1)
    crep1_np = np.ascontiguousarray((counts.T / QS).astype(NP_BF16).reshape(1, R * N))

    in_maps = []
    for k in range(NCORES):
        sl = slice(k * P, (k + 1) * P)
        wpos_k = np.ascontiguousarray(
            Wpq[:, :, sl].reshape(N, DC, P, P).transpose(2, 0, 1, 3)
            .reshape(P, N * DC * P)
        )
        wdep_k = np.ascontiguousarray(
            Wdq[:, :, sl].reshape(R, DC, P, P).transpose(2, 0, 1, 3)
            .reshape(P, R * DC * P)
        )
        baug_k = np.empty((KAUG, P), np.float32)
        baug_k[:R] = b_dep[:, sl]
        baug_k[R] = c_dep
        baug_k[R + 1] = c_pos
        caug_k = np.empty((KAUG, N), np.float32)
        caug_k[:R] = counts.T
        caug_k[R] = sx * csum
        caug_k[R + 1] = sx
        bposT_k = np.ascontiguousarray(b_pos[:, sl].T)

        in_maps.append(
            dict(wpos=wpos_k, wdep=wdep_k, xtf=xtf_np, xtl=xtl_np,
                 crep1=crep1_np, baug=baug_k, caug=caug_k, bposT=bposT_k)
        )
    return in_maps


def _run(in_maps, trace=False):
    nc = _get_program()
    return run_bass_kernel_spmd(nc, in_maps, list(range(NCORES)), trace=trace)


def _assemble(res):
    out_T = np.concatenate([res.results[k]["out_T"] for k in range(NCORES)], axis=0)
    return np.ascontiguousarray(out_T.T)


def kernel(x, W_pos, b_pos, W_dep, b_dep, edge_token, edge_type):
    in_maps = _prepare_in_maps(x, W_pos, b_pos, W_dep, b_dep, edge_token, edge_type)
    res = _run(in_maps, trace=False)
    return _assemble(res)


def kernel_traced(x, W_pos, b_pos, W_dep, b_dep, edge_token, edge_type):
    """Like kernel() but with NTFF profiling; returns (output, BassKernelResults)."""
    in_maps = _prepare_in_maps(x, W_pos, b_pos, W_dep, b_dep, edge_token, edge_type)
    res = _run(in_maps, trace=True)
    return _assemble(res), res


def install_ntff_shim():
    """The agent image's antenv lacks axon_hooks; recreate it from the boot
    module's ctypes NTFF driver so run_bass_kernel_spmd(trace=True) can
    capture a neuron-profile. Test-only; kernel() never needs this."""
    import sys
    import types

    try:
        from antenv.axon_hooks import get_axon_ntff_profile_hook  # noqa: F401
        return
    except ImportError:
        pass
    from trn_agent_boot.trn_boot import _ntff_profile_via_ctypes

    hook = _ntff_profile_via_ctypes("/opt/axon/libaxon_pjrt.so")
    mod = types.ModuleType("antenv.axon_hooks")
    mod._hook = hook
    mod.get_axon_ntff_profile_hook = lambda: mod._hook
    mod.set_axon_ntff_profile_hook = lambda h: setattr(mod, "_hook", h)
    sys.modules["antenv.axon_hooks"] = mod


# revision 10
# speedup vs baseline: 4.2035x; 1.0324x over previous
"""Trainium2 Bass kernel for the GCNN layer (nn_GCNNLayer_71536975282326).

out = relu( einsum('nd,nde->ne', x, W_pos) + b_pos
            + einsum('nre,nr->ne', einsum('nd,rde->nre', x, W_dep), counts)
            + counts @ b_dep )
with counts[n,r] = #edges (token n, type r).

v4 strategy (8 NeuronCores, SPMD, one program) — the problem is HBM-bound
(242 MiB of f32 weights), so the levers are weight bytes and overlap:

  - All weights quantized host-side to fp8 e3m4, mean-centered:
    Wq = round_e3m4((W - c) * 32); the 1/32 is folded into the bf16 moving
    operands, the rank-1 centering correction c*sum_d(x) rides the bias
    matmul.  End-to-end scale-relative error ~3.3e-3 (gate 2e-2) and 4x
    less DMA than f32.
  - Sharding is by OUTPUT e-COLUMN: core k holds e-columns 128k..128k+127
    of EVERY W_pos[n] and W_dep[r] (same total bytes as type-sharding) and
    computes its 128 out_T rows COMPLETELY — no ReduceScatter, no
    AllGather, no cross-core synchronization at all.  v2's collective tail
    (44 us) becomes a 3 us DVE+DMA epilogue.
  - Every weight matmul uses the W 128x128 block as the STATIONARY operand
    (fp8 -> compiler-automatic fast weight load, ~25 ns/block) and a thin
    bf16 moving operand: the token's x column (self, width 1, out column =
    token) or the counts-scaled x (dep, width 150).  Everything accumulates
    in ONE PSUM bank [128 e, 150 tokens].
  - counts*x moving operands: one DVE tensor_mul per 4-type group with
    both inputs as stride-0 broadcast views (x repeated over types, counts
    row repeated over d-chunks); the counts row is replicated across
    partitions once by a GpSimd partition_broadcast.
  - Bias: one K=94 f32 matmul (92 counts rows + 2 centering rows), the
    bank's single start=True writer; b_pos lands in the DVE epilogue.
  - Weights pre-tiled on host so every DMA line is 4-10 KiB contiguous:
    one dma_start per 0.5-1.25 MiB unit = 128 descriptors.
"""

import numpy as np
import ml_dtypes

import concourse.bass as bass
import concourse.tile as tile
from concourse import bacc, mybir
from concourse.bass_utils import run_bass_kernel_spmd

N, D, R = 150, 1024, 92
NCORES = 8
P = 128
DC = D // P            # 8 contraction (d) chunks
SELF_G = 5             # tokens per self DMA unit
SELF_UNITS = N // SELF_G   # 15
DEP_G = 2              # types per dep DMA unit
BCAST_G = 4            # types per partition_broadcast chunk
DEP_UNITS = R // DEP_G     # 23
KAUG = R + 2           # 92 counts rows + 2 centering rows
QS = 32.0
F32 = mybir.dt.float32
BF16 = mybir.dt.bfloat16
F8 = mybir.dt.float8e3

NP_BF16 = ml_dtypes.bfloat16
NP_F8 = ml_dtypes.float8_e3m4

_PROG = None


def _build_program():
    nc = bacc.Bacc("TRN2", target_bir_lowering=False, debug=False, num_devices=NCORES)

    # pre-tiled per-core weight slices (e-columns 128k..128k+127):
    # wpos[p, ((n c)) * 128 + e], wdep[p, ((r c)) * 128 + e]
    wpos = nc.dram_tensor("wpos", [P, N * DC * P], F8, kind="ExternalInput")
    wdep = nc.dram_tensor("wdep", [P, R * DC * P], F8, kind="ExternalInput")
    xtf = nc.dram_tensor("xtf", [P, DC * N], BF16, kind="ExternalInput")
    xtl = nc.dram_tensor("xtl", [P, DC * N], BF16, kind="ExternalInput")  # x^T/QS
    crep1 = nc.dram_tensor("crep1", [1, R * N], BF16, kind="ExternalInput")
    baug = nc.dram_tensor("baug", [KAUG, P], F32, kind="ExternalInput")
    caug = nc.dram_tensor("caug", [KAUG, N], F32, kind="ExternalInput")
    bposT = nc.dram_tensor("bposT", [P, N], F32, kind="ExternalInput")
    out_T = nc.dram_tensor("out_T", [P, N], F32, kind="ExternalOutput")

    with tile.TileContext(nc) as tc:
        with (
            tc.tile_pool(name="constp", bufs=1) as constp,
            tc.tile_pool(name="mainps", bufs=1, space=bass.MemorySpace.PSUM) as mainps,
            tc.tile_pool(name="fin", bufs=2) as fin,
        ):
            baug_t = constp.tile([KAUG, P], F32)
            nc.sync.dma_start(out=baug_t[:], in_=baug[:])
            caug_t = constp.tile([KAUG, N], F32)
            nc.sync.dma_start(out=caug_t[:], in_=caug[:])
            crep1_t = constp.tile([1, R * N], BF16)
            nc.scalar.dma_start(out=crep1_t[:], in_=crep1[:])
            xtf_t = constp.tile([P, DC * N], BF16)
            nc.scalar.dma_start(out=xtf_t[:], in_=xtf[:])
            bposT_t = constp.tile([P, N], F32)
            nc.scalar.dma_start(out=bposT_t[:], in_=bposT[:])
            # x^T/QS for the self moving operand: derived on DVE, saves a load
            xtl_t = constp.tile([P, DC * N], BF16)
            nc.vector.tensor_scalar_mul(xtl_t[:], xtf_t[:], 1.0 / QS)
            # counts/QS replicated across partitions in per-group chunks so
            # the first dep unit's xs only waits ~1us, not a 20us monolith
            crep_t = constp.tile([P, R * N], BF16)
            for g in range(R // BCAST_G):
                nc.gpsimd.partition_broadcast(
                    crep_t[:, g * BCAST_G * N : (g + 1) * BCAST_G * N],
                    crep1_t[:, g * BCAST_G * N : (g + 1) * BCAST_G * N],
                )

            acc = mainps.tile([P, 512], F32, name="acc", tag="acc")
            # bias + centering corrections: the bank's single start=True writer
            nc.tensor.matmul(
                acc[:, 0:N], baug_t[:], caug_t[:], start=True, stop=False,
            )

            with (
                tc.tile_pool(name="wpool", bufs=9) as wpool,
                tc.tile_pool(name="xspool", bufs=7) as xspool,
            ):
                # gpsimd is reserved for the partition_broadcast chain; the
                # W stream triggers rotate over the two HWDGE queues
                engs = [nc.sync, nc.scalar]

                def self_unit(g, eng, stop_last):
                    wt = wpool.tile([P, SELF_G * DC * P], F8, tag="w", name=f"ws{g}")
                    eng.dma_start(
                        out=wt[:],
                        in_=wpos[:, g * SELF_G * DC * P : (g + 1) * SELF_G * DC * P],
                    )
                    for j in range(SELF_G):
                        n = g * SELF_G + j
                        for c in range(DC):
                            nc.tensor.matmul(
                                acc[:, n : n + 1],
                                wt[:, (j * DC + c) * P : (j * DC + c + 1) * P],
                                xtl_t[:, c * N + n : c * N + n + 1],
                                start=False,
                                stop=stop_last and j == SELF_G - 1 and c == DC - 1,
                            )

                def dep_unit(g, eng, stop_last):
                    wt = wpool.tile([P, DEP_G * DC * P], F8, tag="w", name=f"wd{g}")
                    eng.dma_start(
                        out=wt[:],
                        in_=wdep[:, g * DEP_G * DC * P : (g + 1) * DEP_G * DC * P],
                    )
                    # xs[(tr c n)] = x^T[(c n)] * counts[type]/QS, both broadcast
                    xst = xspool.tile([P, DEP_G * DC * N], BF16, tag="xs", name=f"xs{g}")
                    nc.vector.tensor_mul(
                        xst[:].rearrange("p (t c n) -> p t c n", t=DEP_G, c=DC),
                        xtf_t[:, None, :]
                        .rearrange("p t (c n) -> p t c n", c=DC)
                        .broadcast_to([P, DEP_G, DC, N]),
                        crep_t[:, g * DEP_G * N : (g + 1) * DEP_G * N, None]
                        .rearrange("p (t n) c -> p t c n", t=DEP_G)
                        .broadcast_to([P, DEP_G, DC, N]),
                    )
                    for t in range(DEP_G):
                        for c in range(DC):
                            nc.tensor.matmul(
                                acc[:, 0:N],
                                wt[:, (t * DC + c) * P : (t * DC + c + 1) * P],
                                xst[:, (t * DC + c) * N : (t * DC + c + 1) * N],
                                start=False,
                                stop=stop_last and t == DEP_G - 1 and c == DC - 1,
                            )

                # interleave self (DMA-heavy) and dep (PE+DVE-heavy) units;
                # two self units first (gives the DVE xs pipeline lead time),
                # and a dep unit last so the stop lands on a full-width MM
                sched = [("s", 0), ("s", 1)]
                si, di = 2, 0
                while si < SELF_UNITS or di < DEP_UNITS - 1:
                    if di >= DEP_UNITS - 1 or (
                        si < SELF_UNITS
                        and (si - 2) * (DEP_UNITS - 1) <= di * (SELF_UNITS - 2)
                    ):
                        sched.append(("s", si)); si += 1
                    else:
                        sched.append(("d", di)); di += 1
                sched.append(("d", DEP_UNITS - 1))

                for u, (kind, idx) in enumerate(sched):
                    eng = engs[u % len(engs)]
                    last = u == len(sched) - 1
                    if kind == "s":
                        self_unit(idx, eng, last)
                    else:
                        dep_unit(idx, eng, last)

            # ---- epilogue: out = relu(acc + b_pos^T), straight to HBM ----
            oc = fin.tile([P, N], F32, tag="oc")
            nc.vector.scalar_tensor_tensor(
                oc[:], acc[:, 0:N], 0.0, bposT_t[:],
                mybir.AluOpType.add, mybir.AluOpType.add,
            )
            nc.vector.tensor_scalar_max(oc[:], oc[:], 0.0)
            nc.sync.dma_start(out=out_T[:], in_=oc[:])

    nc.compile()
    return nc


def _get_program():
    global _PROG
    if _PROG is None:
        _PROG = _build_program()
    return _PROG


def _prepare_in_maps(x, W_pos, b_pos, W_dep, b_dep, edge_token, edge_type):
    x = np.asarray(x, dtype=np.float32)
    W_pos = np.asarray(W_pos, dtype=np.float32)
    b_pos = np.asarray(b_pos, dtype=np.float32)
    W_dep = np.asarray(W_dep, dtype=np.float32)
    b_dep = np.asarray(b_dep, dtype=np.float32)
    edge_token = np.asarray(edge_token)
    edge_type = np.asarray(edge_type)

    counts = np.zeros((N, R), np.float32)
    np.add.at(counts, (edge_token, edge_type), 1.0)

    c_pos = float(W_pos.max() + W_pos.min()) / 2.0
    c_dep = float(W_dep.max() + W_dep.min()) / 2.0
    Wpq = ((W_pos - c_pos) * QS).astype(NP_F8)   # [N, D, D] fp8
    Wdq = ((W_dep - c_dep) * QS).astype(NP_F8)   # [R, D, D] fp8

    xb = x.astype(NP_BF16)
    xbf = xb.astype(np.float32)
    xT16 = np.ascontiguousarray(xb.T)
    xtf_np = np.ascontiguousarray(
        xT16.reshape(DC, P, N).transpose(1, 0, 2).reshape(P, DC * N)
    )
    xtl_np = np.ascontiguousarray(
        (xbf.T / QS).astype(NP_BF16).reshape(DC, P, N).transpose(1, 0, 2)
        .reshape(P, DC * N)
    )
    sx = xbf.sum(axis=1)
    csum = counts.sum(axis=1)
    crep1_np = np.ascontiguousarray((counts.T / QS).astype(NP_BF16).reshape(1, R * N))

    in_maps = []
    for k in range(NCORES):
        sl = slice(k * P, (k + 1) * P)
        wpos_k = np.ascontiguousarray(
            Wpq[:, :, sl].reshape(N, DC, P, P).transpose(2, 0, 1, 3)
            .reshape(P, N * DC * P)
        )
        wdep_k = np.ascontiguousarray(
            Wdq[:, :, sl].reshape(R, DC, P, P).transpose(2, 0, 1, 3)
            .reshape(P, R * DC * P)
        )
        baug_k = np.empty((KAUG, P), np.float32)
        baug_k[:R] = b_dep[:, sl]
        baug_k[R] = c_dep
        baug_k[R + 1] = c_pos
        caug_k = np.empty((KAUG, N), np.float32)
        caug_k[:R] = counts.T
        caug_k[R] = sx * csum
        caug_k[R + 1] = sx
        bposT_k = np.ascontiguousarray(b_pos[:, sl].T)

        in_maps.append(
            dict(wpos=wpos_k, wdep=wdep_k, xtf=xtf_np, xtl=xtl_np,
                 crep1=crep1_np, baug=baug_k, caug=caug_k, bposT=bposT_k)
        )
    return in_maps


def _run(in_maps, trace=False):
    nc = _get_program()
    return run_bass_kernel_spmd(nc, in_maps, list(range(NCORES)), trace=trace)


def _assemble(res):
    out_T = np.concatenate([res.results[k]["out_T"] for k in range(NCORES)], axis=0)
    return np.ascontiguousarray(out_T.T)


def kernel(x, W_pos, b_pos, W_dep, b_dep, edge_token, edge_type):
    in_maps = _prepare_in_maps(x, W_pos, b_pos, W_dep, b_dep, edge_token, edge_type)
    res = _run(in_maps, trace=False)
    return _assemble(res)


def kernel_traced(x, W_pos, b_pos, W_dep, b_dep, edge_token, edge_type):
    """Like kernel() but with NTFF profiling; returns (output, BassKernelResults)."""
    in_maps = _prepare_in_maps(x, W_pos, b_pos, W_dep, b_dep, edge_token, edge_type)
    res = _run(in_maps, trace=True)
    return _assemble(res), res


def install_ntff_shim():
    """The agent image's antenv lacks axon_hooks; recreate it from the boot
    module's ctypes NTFF driver so run_bass_kernel_spmd(trace=True) can
    capture a neuron-profile. Test-only; kernel() never needs this."""
    import sys
    import types

    try:
        from antenv.axon_hooks import get_axon_ntff_profile_hook  # noqa: F401
        return
    except ImportError:
        pass
    from trn_agent_boot.trn_boot import _ntff_profile_via_ctypes

    hook = _ntff_profile_via_ctypes("/opt/axon/libaxon_pjrt.so")
    mod = types.ModuleType("antenv.axon_hooks")
    mod._hook = hook
    mod.get_axon_ntff_profile_hook = lambda: mod._hook
    mod.set_axon_ntff_profile_hook = lambda h: setattr(mod, "_hook", h)
    sys.modules["antenv.axon_hooks"] = mod
